# revision 49
# baseline (speedup 1.0000x reference)
"""BiLSTM-CRF NLL kernel for 8 TRN2 NeuronCores.

Sharding: data-parallel over batch. B=128 split into 8 shards of 16
sentences; each core runs both LSTM directions, the fc projection, the
CRF forward pass (exp-domain, renorm every R=8 steps, capture at len-1),
and the gold-path score for its shard.

v2 design (vs baseline): the input projection W_ih@emb(x)+bias is folded
into the per-step PSUM accumulation (no preD DRAM round-trip, no DVE
pre-add); embeddings are gathered and transposed on the HOST into xembT
[256, T*B] bf16 (pure relayout, no FLOPs) and DMA'd once into SBUF; the
per-step elementwise chain runs full-width per direction (sig [P,192],
tanh [P,64]) instead of 2x32-col halves; bias enters via 1-row matmuls;
dir-b masking keeps a separate recurrent h (hcur) + 2 copy_predicated,
and hs slots hold unmasked h (garbage beyond len is masked downstream).
CRF: emissions are exp'd with a -SHIFT bias to keep the exp-domain
products in f32 range (logZ corrected by +SHIFT*len at the end), the
alpha history is written in-place over expem, one [12,16] matmul per
step.

Layouts (per core, B=16, T=256):
  gates^T  [4H=2048, B] as 16 m-tiles [128, 16]; one PSUM tile [128, 512]
           holds both dirs (f: cols 0-255, b: 256-511), gate order i|f|o|g
  h^T      slot t of hs [P, (T+1)*128]: cols t*128+[f: 4k x 16b | b: same]
  xembT    [P, 2*T*B] bf16: k-tile k at cols k*T*B, token (t,b) at t*16+b
  emis^T   [12, T*B] f32, col t*16+b
"""

import os
import numpy as np
import ml_dtypes

import concourse.bass as bass
import concourse.bacc as bacc
import concourse.mybir as mybir
import concourse.tile as tile
from concourse.bass import AP

F32 = mybir.dt.float32
BF16 = mybir.dt.bfloat16
I32 = mybir.dt.int32
U8 = mybir.dt.uint8
F8 = mybir.dt.float8e4
DR = mybir.MatmulPerfMode.DoubleRow
MUL = mybir.AluOpType.mult
ADD = mybir.AluOpType.add
SUB = mybir.AluOpType.subtract
X = mybir.AxisListType.X
SIG = mybir.ActivationFunctionType.Sigmoid
TANH = mybir.ActivationFunctionType.Tanh
EXP = mybir.ActivationFunctionType.Exp
LN = mybir.ActivationFunctionType.Ln

P = 128
B = 16            # batch per core
H = 512
E = 256
G = 2048          # 4H
K = 12
START, STOP = 10, 11
R = 8             # CRF renorm period
SHIFT = 3.0       # exp-domain shift: expem = exp(emis + fcb - SHIFT)
NCORES = 8

T = int(os.environ.get("BASS_LSTM_T", "256"))
SKIP = set(os.environ.get("BASS_SKIP", "").split(","))
DEBUG = os.environ.get("BASS_DEBUG", "") == "1"
NE = T // R
MCH = min(32, T)  # mask chunk steps


def fv(t, off, pat):
    """Free-dim view of a contiguous [P, F] tile: keep partition pair, replace
    free dims with `pat` (list of [step, count]) at element offset `off`."""
    base = t[:] if not isinstance(t, AP) else t
    part = list(base.ap[0])
    return AP(base.tensor, base.offset + off, [part] + [list(p) for p in pat])


def build(nc):
    dirs = ("f", "b")
    dt = {}

    def din(name, shape, dtype):
        dt[name] = nc.dram_tensor(name, shape, dtype, kind="ExternalInput")
        return dt[name]

    for d in dirs:
        din(f"wihT_{d}", [E, G], F8)
        din(f"whhT_{d}", [H, G], F8)
        din(f"b16_{d}", [16, P], BF16)
        din(f"fcWT_{d}", [H, K], F8)
    din("onehot", [16, 256], BF16)
    din("xembT", [P, 2 * T * B], F8)
    din("h0T", [P, 2 * 64], F8)
    din("c0T", [P, 2 * 64], BF16)
    din("mask_b", [T, P, 64], U8)
    din("imask_b", [T, P, 64], U8)
    din("transT", [K, K], F32)
    din("trans", [K, K], F32)
    din("fcb", [K], F32)
    din("fcbs", [K], F32)
    din("corrL", [B], F32)
    din("a0", [K, B], F32)
    din("msel", [K, T * B], F32)
    din("maskep", [NE * B], F32)
    din("sel", [K, T * B], F32)
    din("counts", [B, 144], F32)
    din("cntb", [B, K], F32)

    nll_o = nc.dram_tensor("nll", [B], F32, kind="ExternalOutput")
    dlogz_o = nc.dram_tensor("dbg_logz", [B], F32, kind="ExternalOutput")
    dgold_o = nc.dram_tensor("dbg_gold", [B], F32, kind="ExternalOutput")
    if DEBUG:
        demis_o = nc.dram_tensor("dbg_emis", [K, T * B], F32, kind="ExternalOutput")
        dhs_o = nc.dram_tensor("dbg_hs", [P, (T + 1) * 128], BF16, kind="ExternalOutput")
    scr16 = nc.dram_tensor("scr16", [B], F32)

    with tile.TileContext(nc) as tc:
        with tc.tile_pool(name="persist", bufs=1) as pp:
            fcw = {d: pp.tile([P, 4 * K], F8, name=f"fcw{d}", tag=f"fcw{d}") for d in dirs}
            ones16 = pp.tile([1, B], BF16, tag="ones16")
            hsf = pp.tile([P, (T + 1) * 64], F8, tag="hsf")
            hsb = pp.tile([P, (T + 1) * 64], F8, tag="hsb")
            cgf = pp.tile([P, 128], BF16, tag="cgf")
            cgb = pp.tile([P, 128], BF16, tag="cgb")
            emisT = pp.tile([K, T * B], F32, tag="emisT")
            expem = pp.tile([K, T * B], F32, tag="expem")
            fcbs_p = pp.tile([K, 1], F32, tag="fcbs_p")

            nc.gpsimd.memset(ones16[:], 1.0)
            # spread initial DMAs across issue queues
            issuers = [nc.gpsimd, nc.sync, nc.scalar]
            nq = 0

            def dma(dst, src):
                nonlocal nq
                issuers[nq % len(issuers)].dma_start(dst, src)
                nq += 1

            # ---- recurrence (input proj + bias folded into PSUM) ----
            TB = T * B
            with tc.tile_pool(name="rec_w", bufs=1) as wp, \
                 tc.tile_pool(name="rec_sbuf", bufs=6) as rp, \
                 tc.tile_pool(name="rec_mask", bufs=2) as mp, \
                 tc.tile_pool(name="rec_psum", bufs=2, space="PSUM") as rpp, \
                 tc.tile_pool(name="rec_psumg", bufs=1, space="PSUM") as rppg, \
                 tc.tile_pool(name="fc_psum", bufs=2, space="PSUM") as fpp:
                whh = {d: wp.tile([P, 4 * 16 * P], F8, name=f"whh{d}", tag=f"whh{d}") for d in dirs}
                wih = {d: wp.tile([P, 2 * 16 * P], F8, name=f"wih{d}", tag=f"wih{d}") for d in dirs}
                brow = {d: wp.tile([1, G], BF16, name=f"brow{d}", tag=f"brow{d}") for d in dirs}
                xemb = wp.tile([P, 2 * T * B], F8, tag="xemb")
                for d in dirs:
                    for k in range(4):
                        dma(whh[d][:, k * 16 * P:(k + 1) * 16 * P],
                            dt[f"whhT_{d}"].ap()[k * P:(k + 1) * P, :])
                        dma(fcw[d][:, k * K:(k + 1) * K],
                            dt[f"fcWT_{d}"].ap()[k * P:(k + 1) * P, :])
                    for k in range(2):
                        dma(wih[d][:, k * 16 * P:(k + 1) * 16 * P],
                            dt[f"wihT_{d}"].ap()[k * P:(k + 1) * P, :])
                    dma(brow[d][:], AP(dt[f"b16_{d}"], 0, [[1, 1], [1, G]]))
                NXQ = 4  # xemb in quarters
                for q in range(NXQ):
                    w = 2 * T * B // NXQ
                    dma(xemb[:, q * w:(q + 1) * w],
                        dt["xembT"].ap()[:, q * w:(q + 1) * w])
                dma(hsf[:, 0:64], dt["h0T"].ap()[:, 0:64])
                dma(hsb[:, 0:64], dt["h0T"].ap()[:, 64:128])
                dma(cgf[:, 0:64], dt["c0T"].ap()[:, 0:64])
                dma(cgb[:, 0:64], dt["c0T"].ap()[:, 64:128])
                dma(fcbs_p[:], AP(dt["fcbs"], 0, [[1, K], [1, 1]]))
                ps_tiles = {}

                def _in_mms(ps, d, blk, m, first):
                    # bias + input-projection matmuls for one m-tile; exactly
                    # one start=True per PSUM bank per step (start zeroes the
                    # whole 2KB zero region)
                    o = (m % 12) * 16 if m < 12 else (m - 12) * 16
                    nc.tensor.matmul(
                        ps[:, o:o + 16],
                        brow[d][0:1, m * P:(m + 1) * P], ones16[0:1, :],
                        start=first, stop=False)
                    nc.tensor.matmul(
                        ps[:, o:o + 16],
                        fv(wih[d], m * P, [[16 * P, 2], [1, P]]),
                        fv(xemb, blk * B, [[TB, 2], [1, B]]),
                        start=False, stop=False, perf_mode=DR)

                def emit_in(t):
                    # per dir: ifo gates and g gates in SEPARATE full-bank
                    # PSUM tiles — PSUM deps are bank-granular, so this lets
                    # tanh(g) start before the ifo matmuls finish
                    pst = {}
                    for d in dirs:
                        psI = rpp.tile([P, 512], F32, name=f"psI{d}", tag=f"psI{d}")
                        psG = rppg.tile([P, 512], F32, name=f"psG{d}", tag=f"psG{d}")
                        pst[d] = (psI, psG)
                        blk = t if d == "f" else T - 1 - t
                        for m in range(12, 16):
                            _in_mms(psG, d, blk, m, m == 12)
                        for m in range(12):
                            _in_mms(psI, d, blk, m, m == 0)
                    ps_tiles[t] = pst

                def whh_mms(pst, d, t):
                    psI, psG = pst[d]
                    rhst = hsf if d == "f" else hsb
                    roff = t * 64
                    for m in list(range(12, 16)) + list(range(12)):
                        ps = psG if m >= 12 else psI
                        o = (m - 12) * 16 if m >= 12 else m * 16
                        last = (m == 15) or (m == 11)
                        for kp in range(2):
                            nc.tensor.matmul(
                                ps[:, o:o + 16],
                                fv(whh[d], (2 * kp * 16 + m) * P, [[16 * P, 2], [1, P]]),
                                fv(rhst, roff + kp * 32, [[16, 2], [1, B]]),
                                start=False, stop=(last and kp == 1), perf_mode=DR)

                maskst = {"ch": None}

                bst = {}

                def elem_f1(pst, t):
                    psI, psG = pst["f"]
                    nc.scalar.activation(cgf[:, 64:128], psG[:, 0:64], TANH)
                    sifo = rp.tile([P, 192], BF16, tag="sifof")
                    nc.scalar.activation(sifo[:], psI[:, 0:192], SIG)
                    return (sifo,)

                def elem_f2(sifo, t):
                    # t12 = [sig_f | sig_i] * [c | tanh(g)] in one op
                    t12 = rp.tile([P, 128], BF16, tag="t12f")
                    nc.vector.tensor_tensor(
                        t12[:], fv(sifo, 64, [[-64, 2], [1, 64]]), cgf[:], op=MUL)
                    nc.vector.tensor_tensor(cgf[:, 0:64], t12[:, 0:64],
                                            t12[:, 64:128], op=ADD)
                    tch = rp.tile([P, 64], BF16, tag="tcf")
                    nc.scalar.activation(tch[:], cgf[:, 0:64], TANH)
                    hslot = hsf[:, (t + 1) * 64: (t + 1) * 64 + 64]
                    nc.vector.tensor_tensor(hslot, sifo[:, 128:192], tch[:], op=MUL)

                def load_masks(t0):
                    n = min(MCH, T - t0)
                    mk = mp.tile([P, MCH * 64], U8, name="maskch", tag="maskch")
                    imk = mp.tile([P, MCH * 64], U8, name="imaskch", tag="imaskch")
                    nc.gpsimd.dma_start(
                        mk[:, 0:n * 64], AP(dt["mask_b"], t0 * P * 64,
                                            [[64, P], [P * 64, n], [1, 64]]))
                    nc.gpsimd.dma_start(
                        imk[:, 0:n * 64], AP(dt["imask_b"], t0 * P * 64,
                                             [[64, P], [P * 64, n], [1, 64]]))
                    maskst[t0] = (mk, imk)

                def elem_b1(pst, t):
                    ps = pst["b"]
                    tl = t % MCH
                    if tl == 0:
                        if t + MCH < T:
                            load_masks(t + MCH)
                        maskst["cur"] = maskst.pop(t)
                    psI, psG = ps
                    nc.scalar.activation(cgb[:, 64:128], psG[:, 0:64], TANH)
                    sifo = rp.tile([P, 192], BF16, tag="sifob")
                    nc.scalar.activation(sifo[:], psI[:, 0:192], SIG)
                    t12 = rp.tile([P, 128], BF16, tag="t12b")
                    nc.vector.tensor_tensor(
                        t12[:], fv(sifo, 64, [[-64, 2], [1, 64]]), cgb[:], op=MUL)
                    cn = rp.tile([P, 64], BF16, tag="cn")
                    nc.vector.tensor_tensor(cn[:], t12[:, 0:64], t12[:, 64:128], op=ADD)
                    mk, imk = maskst["cur"]
                    om = rp.tile([P, 64], BF16, tag="om")
                    nc.vector.tensor_tensor(om[:], sifo[:, 128:192],
                                            mk[:, tl * 64:(tl + 1) * 64], op=MUL)
                    u2 = rp.tile([P, 64], BF16, tag="u2")
                    nc.vector.tensor_tensor(u2[:], hsb[:, t * 64:(t + 1) * 64],
                                            imk[:, tl * 64:(tl + 1) * 64], op=MUL)
                    bst["sifo"], bst["cn"], bst["tl"] = sifo, cn, tl
                    bst["om"], bst["u2"] = om, u2

                def elem_b2(t):
                    sifo, cn, tl = bst["sifo"], bst["cn"], bst["tl"]
                    om, u2 = bst["om"], bst["u2"]
                    tch = rp.tile([P, 64], BF16, tag="tcb")
                    nc.scalar.activation(tch[:], cn[:], TANH)
                    # h_next = mask*sig_o*tanh(c_new) + (1-mask)*h_prev,
                    # written straight into the hs slot (whh_b reads it)
                    b1 = rp.tile([P, 64], BF16, tag="b1")
                    nc.vector.tensor_tensor(b1[:], om[:], tch[:], op=MUL)
                    hslot = hsb[:, (t + 1) * 64: (t + 1) * 64 + 64]
                    nc.vector.tensor_tensor(hslot, b1[:], u2[:], op=ADD)
                    mk = maskst["cur"][0][:, tl * 64:(tl + 1) * 64]
                    nc.vector.copy_predicated(cgb[:, 0:64], mk, cn[:])

                # half-step stagger: dir b's elementwise for step t-1 runs
                # between dir f's matmuls and dir f's elementwise for step t;
                # sig_f/gt_f are emitted before tch_b so the Act queue order
                # matches readiness order
                if "rec" not in SKIP:
                    load_masks(0)
                    emit_in(0)
                for t in range(0 if "rec" in SKIP else T):
                    ps = ps_tiles[t]
                    whh_mms(ps, "f", t)
                    if t > 0:
                        elem_b1(ps_tiles.pop(t - 1), t - 1)
                    sg = elem_f1(ps, t)
                    if t > 0:
                        elem_b2(t - 1)
                    if t + 1 < T:
                        emit_in(t + 1)
                    whh_mms(ps, "b", t)
                    elem_f2(*sg, t)
                if "rec" not in SKIP:
                    elem_b1(ps_tiles.pop(T - 1), T - 1)
                    elem_b2(T - 1)

                # fc chunks, emitted in readiness order so they overlap the
                # recurrence tail (chunk c ready at step max(f-end, b-end))
                NCH = max(1, T * B // 512)
                SPC = T // NCH
                order = sorted(range(NCH),
                               key=lambda c: max(SPC * c + SPC - 1, T - 1 - SPC * c))
                for c in (order if "fc" not in SKIP else []):
                    psf = fpp.tile([K, 512], F32, name="psf", tag="psf")
                    for d in dirs:
                        for k in range(4):
                            if d == "f":
                                rhs = fv(hsf, (c * SPC + 1) * 64 + k * 16,
                                         [[64, SPC], [1, B]])
                            else:
                                rhs = fv(hsb, (T - c * SPC) * 64 + k * 16,
                                         [[-64, SPC], [1, B]])
                            nc.tensor.matmul(
                                psf[:, 0:SPC * B], fcw[d][:, k * K:(k + 1) * K], rhs,
                                start=(d == "f" and k == 0), stop=(d == "b" and k == 3))
                    sl = slice(c * SPC * B, (c + 1) * SPC * B)
                    nc.vector.tensor_copy(emisT[:, sl], psf[:, 0:SPC * B])

            # ---- CRF chain, gold, capture ----
            with tc.tile_pool(name="tail_sbuf", bufs=2) as cp, \
                 tc.tile_pool(name="tail_persist", bufs=1) as cpr, \
                 tc.tile_pool(name="tail_psum", bufs=2, space="PSUM") as cpp:
                transTs = cpr.tile([K, K], F32, tag="transTs")
                nc.gpsimd.dma_start(transTs[:], dt["transT"].ap()[:])
                ET = cpr.tile([K, K], F32, tag="ET")
                nc.scalar.activation(ET[:], transTs[:], EXP)
                Estop = cpr.tile([K, 1], F32, tag="Estop")
                nc.scalar.activation(Estop[:], transTs[:, STOP:STOP + 1], EXP)
                ones12 = cpr.tile([K, K], F32, tag="ones12")
                nc.gpsimd.memset(ones12[:], 1.0)
                NCH = max(1, T * B // 512)
                SPC = T // NCH
                for c in range(NCH):
                    sl = slice(c * SPC * B, (c + 1) * SPC * B)
                    nc.scalar.activation(expem[:, sl], emisT[:, sl], EXP,
                                         bias=fcbs_p[:, 0:1])
                a0 = cpr.tile([K, B], F32, tag="a0")
                nc.gpsimd.dma_start(a0[:], dt["a0"].ap()[:])
                Lh = cpr.tile([1, NE * B], F32, tag="Lh")
                nc.gpsimd.memset(Lh[:], 0.0)
                selb = cpr.tile([K, T * B], F32, tag="selb")
                nc.gpsimd.dma_start(selb[:], dt["sel"].ap()[:])
                mselb = cpr.tile([K, T * B], F32, tag="mselb")
                nc.gpsimd.dma_start(mselb[:], dt["msel"].ap()[:])

                if DEBUG:
                    nc.gpsimd.dma_start(demis_o.ap()[:], emisT[:])
                    nc.gpsimd.dma_start(
                        AP(dhs_o, 0, [[(T + 1) * 128, P], [1, (T + 1) * 64]]), hsf[:])
                    nc.gpsimd.dma_start(
                        AP(dhs_o, (T + 1) * 64, [[(T + 1) * 128, P], [1, (T + 1) * 64]]), hsb[:])

                # CRF forward chain; renorm factors are computed off-chain
                # and applied one step late so the chain never stalls
                rhs = a0[:, 0:B]
                rS_pend = None
                for t in range(0 if "crf" in SKIP else T):
                    blk = expem[:, t * B:(t + 1) * B]
                    psc = cpp.tile([K, 512], F32, name="psc", tag="psc")
                    nc.tensor.matmul(psc[:, 0:B], ET[:], rhs, start=True, stop=True)
                    nc.vector.tensor_tensor(blk, psc[:, 0:B], blk, op=MUL)
                    if t % R == 0 and t > 0:
                        nc.gpsimd.tensor_tensor(blk, blk, rS_pend[:], op=MUL)
                    rhs = blk
                    if t % R == R - 1 and t < T - 1:
                        j = (t + 1) // R
                        pss = cpp.tile([K, 512], F32, name="pss", tag="pss", bufs=2)
                        nc.tensor.matmul(pss[:, 0:B], ones12[:], blk,
                                         start=True, stop=True)
                        rS_pend = cp.tile([K, B], F32, tag="rS")
                        nc.vector.reciprocal(rS_pend[:], pss[:, 0:B])
                        lnS = cp.tile([1, B], F32, tag="lnS")
                        nc.scalar.activation(lnS[:], pss[0:1, 0:B], LN)
                        nc.vector.tensor_tensor(Lh[:, j * B:(j + 1) * B],
                                                Lh[:, (j - 1) * B:j * B],
                                                lnS[:], op=ADD)

                # gold score (independent of the chain; fills engine gaps)
                tfl = cp.tile([1, 144], F32, tag="tfl")
                nc.gpsimd.dma_start(tfl[:], AP(dt["trans"], 0, [[1, 1], [1, 144]]))
                tfb = cp.tile([B, 144], F32, tag="tfb")
                nc.gpsimd.partition_broadcast(tfb[:], tfl[:])
                cnts = cp.tile([B, 144], F32, tag="cnts")
                nc.gpsimd.dma_start(cnts[:], dt["counts"].ap()[:])
                pr1 = cp.tile([B, 144], F32, tag="pr1")
                nc.vector.tensor_tensor(pr1[:], cnts[:], tfb[:], op=MUL)
                g1 = cp.tile([B, 1], F32, tag="g1")
                nc.vector.tensor_reduce(g1[:], pr1[:], axis=X, op=ADD)
                fcbr = cp.tile([1, K], F32, tag="fcbr")
                nc.gpsimd.dma_start(fcbr[:], AP(dt["fcb"], 0, [[1, 1], [1, K]]))
                fcbb = cp.tile([B, K], F32, tag="fcbb")
                nc.gpsimd.partition_broadcast(fcbb[:], fcbr[:])
                cntbs = cp.tile([B, K], F32, tag="cntbs")
                nc.gpsimd.dma_start(cntbs[:], dt["cntb"].ap()[:])
                pr2 = cp.tile([B, K], F32, tag="pr2")
                nc.vector.tensor_tensor(pr2[:], cntbs[:], fcbb[:], op=MUL)
                g2 = cp.tile([B, 1], F32, tag="g2")
                nc.vector.tensor_reduce(g2[:], pr2[:], axis=X, op=ADD)
                g12 = cp.tile([B, 1], F32, tag="g12")
                nc.vector.tensor_tensor(g12[:], g1[:], g2[:], op=ADD)
                nc.gpsimd.dma_start(AP(scr16, 0, [[1, B], [1, 1]]), g12[:])
                g12r = cp.tile([1, B], F32, tag="g12r")
                nc.gpsimd.dma_start(g12r[:], AP(scr16, 0, [[1, 1], [1, B]]))
                selp = cpr.tile([K, NCH * B], F32, tag="selp")
                for c in range(NCH):
                    sl = slice(c * SPC * B, (c + 1) * SPC * B)
                    nc.vector.tensor_tensor(selb[:, sl], emisT[:, sl],
                                            selb[:, sl], op=MUL)
                    nc.vector.tensor_reduce(
                        selp[:, c * B:(c + 1) * B],
                        fv(selb, c * SPC * B, [[1, B], [B, SPC]]), axis=X, op=ADD)
                g3 = cp.tile([K, B], F32, tag="g3")
                nc.vector.tensor_reduce(g3[:], fv(selp, 0, [[1, B], [B, NCH]]),
                                        axis=X, op=ADD)
                ps3 = cpp.tile([K, 512], F32, name="ps3", tag="pss", bufs=2)
                nc.tensor.matmul(ps3[:, 0:B], ones12[:], g3[:], start=True, stop=True)
                goldT = cp.tile([1, B], F32, tag="goldT")
                nc.vector.tensor_tensor(goldT[:], g12r[:], ps3[0:1, 0:B], op=ADD)
                nc.gpsimd.dma_start(AP(dgold_o, 0, [[1, 1], [1, B]]), goldT[:])

                # capture at t = len-1 (expem holds the alpha history);
                # chunked so it runs as the chain passes each block
                aendp = cpr.tile([K, NCH * B], F32, tag="aendp")
                for c in range(NCH):
                    sl = slice(c * SPC * B, (c + 1) * SPC * B)
                    nc.vector.tensor_tensor(expem[:, sl], expem[:, sl],
                                            mselb[:, sl], op=MUL)
                    nc.vector.tensor_reduce(
                        aendp[:, c * B:(c + 1) * B],
                        fv(expem, c * SPC * B, [[1, B], [B, SPC]]), axis=X, op=ADD)
                aend = cp.tile([K, B], F32, tag="aend")
                nc.vector.tensor_reduce(aend[:], fv(aendp, 0, [[1, B], [B, NCH]]),
                                        axis=X, op=ADD)
                mep = cp.tile([1, NE * B], F32, tag="mep")
                nc.gpsimd.dma_start(mep[:], AP(dt["maskep"], 0, [[1, 1], [1, NE * B]]))
                prod5 = cp.tile([1, NE * B], F32, tag="prod5")
                nc.vector.tensor_tensor(prod5[:], Lh[:], mep[:], op=MUL)
                Lend = cp.tile([1, B], F32, tag="Lend")
                nc.vector.tensor_reduce(Lend[:], fv(prod5, 0, [[1, B], [B, NE]]),
                                        axis=X, op=ADD)
                azs = cp.tile([K, B], F32, tag="azs")
                nc.vector.tensor_scalar(out=azs[:], in0=aend[:], scalar1=Estop[:, 0:1],
                                        scalar2=None, op0=MUL)
                ps2 = cpp.tile([K, 512], F32, name="ps2", tag="pss", bufs=2)
                nc.tensor.matmul(ps2[:, 0:B], ones12[:], azs[:], start=True, stop=True)
                logz0 = cp.tile([1, B], F32, tag="logz0")
                nc.scalar.activation(logz0[:], ps2[0:1, 0:B], LN)
                corr = cp.tile([1, B], F32, tag="corr")
                nc.gpsimd.dma_start(corr[:], AP(dt["corrL"], 0, [[1, 1], [1, B]]))
                logzp = cp.tile([1, B], F32, tag="logzp")
                nc.vector.tensor_tensor(logzp[:], logz0[:], Lend[:], op=ADD)
                logzf = cp.tile([1, B], F32, tag="logzf")
                nc.vector.tensor_tensor(logzf[:], logzp[:], corr[:], op=ADD)
                nc.gpsimd.dma_start(AP(dlogz_o, 0, [[1, 1], [1, B]]), logzf[:])
                nllT = cp.tile([1, B], F32, tag="nllT")
                nc.vector.tensor_tensor(nllT[:], logzf[:], goldT[:], op=SUB)
                nc.gpsimd.dma_start(AP(nll_o, 0, [[1, 1], [1, B]]), nllT[:])
    return nc


_CACHE = {}


def get_program():
    if "nc" not in _CACHE:
        nc = bacc.Bacc("TRN2", target_bir_lowering=False, debug=False,
                       num_devices=NCORES)
        build(nc)
        nc.compile()
        _CACHE["nc"] = nc
    return _CACHE["nc"]


def perm_ifog(w):
    # [4H, ...] rows i,f,g,o -> i,f,o,g
    return np.concatenate([w[0:512], w[512:1024], w[1536:2048], w[1024:1536]], 0)


def host_prep(inputs):
    f32 = np.float32
    bf = ml_dtypes.bfloat16
    f8 = ml_dtypes.float8_e4m3fn
    x = np.asarray(inputs["x"]).astype(np.int64)
    lengths = np.asarray(inputs["lengths"]).astype(np.int64)
    tags = np.asarray(inputs["tags"]).astype(np.int64)
    emb = np.asarray(inputs["embedding"], f32)
    trans = np.asarray(inputs["trans"], f32)
    fcW = np.asarray(inputs["fc_W"], f32)
    fcb = np.asarray(inputs["fc_b"], f32)
    h0 = np.asarray(inputs["h0"], f32)
    c0 = np.asarray(inputs["c0"], f32)

    Wd, Brow = {}, {}
    for d in ("f", "b"):
        wih = perm_ifog(np.asarray(inputs[f"W_ih_{d}"], f32))
        whh = perm_ifog(np.asarray(inputs[f"W_hh_{d}"], f32))
        bi = perm_ifog(np.asarray(inputs[f"b_ih_{d}"], f32)[:, None])[:, 0]
        bh = perm_ifog(np.asarray(inputs[f"b_hh_{d}"], f32)[:, None])[:, 0]
        Wd[d] = (wih.T.astype(f8).copy(), whh.T.astype(f8).copy())
        Brow[d] = (bi + bh).reshape(16, P).astype(bf).copy()
    onehot = (np.arange(256)[None, :] // 16 == np.arange(16)[:, None]).astype(bf)

    fcWT = {"f": fcW[:, :H].T.astype(f8).copy(), "b": fcW[:, H:].T.astype(f8).copy()}
    emb_bf = emb.astype(f8)

    maps = []
    for c in range(NCORES):
        bs = slice(c * B, (c + 1) * B)
        xs = x[bs]            # [16, T]
        ln = lengths[bs]      # [16]
        tg = tags[bs]         # [16, T]
        m = {"trans": trans, "transT": trans.T.astype(f32).copy(),
             "fcb": fcb, "fcbs": (fcb - SHIFT).astype(f32),
             "corrL": (SHIFT * ln).astype(f32)}
        # xembT: [256, T, 16] -> [2, 128, T*16] -> [128, 2*T*16]
        e = emb_bf[xs]                        # [16, T, 256]
        eT = e.transpose(2, 1, 0)             # [256, T, 16]
        arr = eT.reshape(2, P, T * B)
        m["xembT"] = np.ascontiguousarray(
            np.concatenate([arr[0], arr[1]], axis=1))
        m["onehot"] = onehot
        for d in ("f", "b"):
            m[f"wihT_{d}"], m[f"whhT_{d}"] = Wd[d]
            m[f"b16_{d}"] = Brow[d]
            m[f"fcWT_{d}"] = fcWT[d]
        h0T = np.concatenate(
            [h0[di, bs].T.reshape(4, P, B).transpose(1, 0, 2).reshape(P, 64)
             for di in (0, 1)], axis=1)
        c0T = np.concatenate(
            [c0[di, bs].T.reshape(4, P, B).transpose(1, 0, 2).reshape(P, 64)
             for di in (0, 1)], axis=1)
        m["h0T"] = h0T.astype(f8).copy()
        m["c0T"] = c0T.astype(bf).copy()
        # bwd mask: step s processes tau = T-1-s; valid iff tau < len
        tau = (T - 1 - np.arange(T))[:, None]          # [T, 1]
        mk = (tau < ln[None, :]).astype(f32)           # [T, 16]
        mku8 = np.broadcast_to(
            mk[:, None, None, :], (T, P, 4, B)).reshape(T, P, 64).astype(np.uint8)
        m["mask_b"] = mku8.copy()
        m["imask_b"] = (1 - mku8).copy()
        a0 = np.zeros((K, B), f32); a0[START, :] = 1.0
        m["a0"] = a0
        msel = np.zeros((K, T, B), f32)
        msel[:, ln - 1, np.arange(B)] = 1.0
        m["msel"] = msel.reshape(K, T * B)
        mep = np.zeros((NE, B), f32)
        mep[(ln - 1) // R, np.arange(B)] = 1.0
        m["maskep"] = mep.reshape(-1)
        tarange = np.arange(T)[None, :]
        valid = tarange < ln[:, None]                  # [16, T]
        selm = np.zeros((K, T, B), f32)
        jj = np.arange(K)[:, None, None]
        selm[:] = (tg.T[None] == jj) & valid.T[None]
        m["sel"] = np.ascontiguousarray(selm.reshape(K, T * B))
        counts = np.zeros((B, 144), f32)
        cntb = np.zeros((B, K), f32)
        for b in range(B):
            L = int(ln[b])
            prev = START
            for t in range(L):
                nx = int(tg[b, t])
                counts[b, nx * K + prev] += 1
                cntb[b, nx] += 1
                prev = nx
            counts[b, STOP * K + prev] += 1
        m["counts"] = counts
        m["cntb"] = cntb
        maps.append(m)
    return maps


def kernel(**inputs):
    from concourse.bass_utils import run_bass_kernel_spmd
    nc = get_program()
    maps = host_prep(inputs)
    res = run_bass_kernel_spmd(nc, maps, core_ids=list(range(NCORES)))
    out = np.concatenate([r["nll"] for r in res.results]).astype(np.float32)
    kernel.last_results = res
    return out


# revision 50
# speedup vs baseline: 1.0153x; 1.0153x over previous
"""BiLSTM-CRF NLL kernel for 8 TRN2 NeuronCores.

Sharding: data-parallel over batch. B=128 split into 8 shards of 16
sentences; each core runs both LSTM directions, the fc projection, the
CRF forward pass (exp-domain, renorm every R=8 steps, capture at len-1),
and the gold-path score for its shard.

v2 design (vs baseline): the input projection W_ih@emb(x)+bias is folded
into the per-step PSUM accumulation (no preD DRAM round-trip, no DVE
pre-add); embeddings are gathered and transposed on the HOST into xembT
[256, T*B] bf16 (pure relayout, no FLOPs) and DMA'd once into SBUF; the
per-step elementwise chain runs full-width per direction (sig [P,192],
tanh [P,64]) instead of 2x32-col halves; bias enters via 1-row matmuls;
dir-b masking keeps a separate recurrent h (hcur) + 2 copy_predicated,
and hs slots hold unmasked h (garbage beyond len is masked downstream).
CRF: emissions are exp'd with a -SHIFT bias to keep the exp-domain
products in f32 range (logZ corrected by +SHIFT*len at the end), the
alpha history is written in-place over expem, one [12,16] matmul per
step.

Layouts (per core, B=16, T=256):
  gates^T  [4H=2048, B] as 16 m-tiles [128, 16]; one PSUM tile [128, 512]
           holds both dirs (f: cols 0-255, b: 256-511), gate order i|f|o|g
  h^T      slot t of hs [P, (T+1)*128]: cols t*128+[f: 4k x 16b | b: same]
  xembT    [P, 2*T*B] bf16: k-tile k at cols k*T*B, token (t,b) at t*16+b
  emis^T   [12, T*B] f32, col t*16+b
"""

import os
import numpy as np
import ml_dtypes

import concourse.bass as bass
import concourse.bacc as bacc
import concourse.mybir as mybir
import concourse.tile as tile
from concourse.bass import AP

F32 = mybir.dt.float32
BF16 = mybir.dt.bfloat16
I32 = mybir.dt.int32
U8 = mybir.dt.uint8
F8 = mybir.dt.float8e4
DR = mybir.MatmulPerfMode.DoubleRow
MUL = mybir.AluOpType.mult
ADD = mybir.AluOpType.add
SUB = mybir.AluOpType.subtract
X = mybir.AxisListType.X
SIG = mybir.ActivationFunctionType.Sigmoid
TANH = mybir.ActivationFunctionType.Tanh
EXP = mybir.ActivationFunctionType.Exp
LN = mybir.ActivationFunctionType.Ln

P = 128
B = 16            # batch per core
H = 512
E = 256
G = 2048          # 4H
K = 12
START, STOP = 10, 11
R = 8             # CRF renorm period
SHIFT = 3.0       # exp-domain shift: expem = exp(emis + fcb - SHIFT)
NCORES = 8

T = int(os.environ.get("BASS_LSTM_T", "256"))
SKIP = set(os.environ.get("BASS_SKIP", "").split(","))
DEBUG = os.environ.get("BASS_DEBUG", "") == "1"
NE = T // R
MCH = min(32, T)  # mask chunk steps


def fv(t, off, pat):
    """Free-dim view of a contiguous [P, F] tile: keep partition pair, replace
    free dims with `pat` (list of [step, count]) at element offset `off`."""
    base = t[:] if not isinstance(t, AP) else t
    part = list(base.ap[0])
    return AP(base.tensor, base.offset + off, [part] + [list(p) for p in pat])


def build(nc):
    dirs = ("f", "b")
    dt = {}

    def din(name, shape, dtype):
        dt[name] = nc.dram_tensor(name, shape, dtype, kind="ExternalInput")
        return dt[name]

    for d in dirs:
        din(f"wihT_{d}", [E, G], F8)
        din(f"whhT_{d}", [H, G], F8)
        din(f"b16_{d}", [16, P], BF16)
        din(f"fcWT_{d}", [H, K], F8)
    din("onehot", [16, 256], BF16)
    din("xembT", [P, 2 * T * B], F8)
    din("h0T", [P, 2 * 64], F8)
    din("c0T", [P, 2 * 64], BF16)
    din("mask_b", [T, P, 64], U8)
    din("imask_b", [T, P, 64], U8)
    din("transT", [K, K], F32)
    din("trans", [K, K], F32)
    din("fcb", [K], F32)
    din("fcbs", [K], F32)
    din("corrL", [B], F32)
    din("a0", [K, B], F32)
    din("msel", [K, T * B], F32)
    din("maskep", [NE * B], F32)
    din("sel", [K, T * B], F32)
    din("counts", [B, 144], F32)
    din("cntb", [B, K], F32)

    nll_o = nc.dram_tensor("nll", [B], F32, kind="ExternalOutput")
    dlogz_o = nc.dram_tensor("dbg_logz", [B], F32, kind="ExternalOutput")
    dgold_o = nc.dram_tensor("dbg_gold", [B], F32, kind="ExternalOutput")
    if DEBUG:
        demis_o = nc.dram_tensor("dbg_emis", [K, T * B], F32, kind="ExternalOutput")
        dhs_o = nc.dram_tensor("dbg_hs", [P, (T + 1) * 128], BF16, kind="ExternalOutput")
    scr16 = nc.dram_tensor("scr16", [B], F32)

    with tile.TileContext(nc) as tc:
        with tc.tile_pool(name="persist", bufs=1) as pp:
            fcw = {d: pp.tile([P, 4 * K], F8, name=f"fcw{d}", tag=f"fcw{d}") for d in dirs}
            ones16 = pp.tile([1, B], BF16, tag="ones16")
            hsf = pp.tile([P, (T + 1) * 64], F8, tag="hsf")
            hsb = pp.tile([P, (T + 1) * 64], F8, tag="hsb")
            cgf = pp.tile([P, 128], BF16, tag="cgf")
            cgb = pp.tile([P, 128], BF16, tag="cgb")
            emisT = pp.tile([K, T * B], F32, tag="emisT")
            expem = pp.tile([K, T * B], F32, tag="expem")
            fcbs_p = pp.tile([K, 1], F32, tag="fcbs_p")

            nc.gpsimd.memset(ones16[:], 1.0)
            # spread initial DMAs across issue queues
            issuers = [nc.gpsimd, nc.sync, nc.scalar]
            nq = 0

            def dma(dst, src):
                nonlocal nq
                issuers[nq % len(issuers)].dma_start(dst, src)
                nq += 1

            # ---- recurrence (input proj + bias folded into PSUM) ----
            TB = T * B
            with tc.tile_pool(name="rec_w", bufs=1) as wp, \
                 tc.tile_pool(name="rec_sbuf", bufs=6) as rp, \
                 tc.tile_pool(name="rec_mask", bufs=2) as mp, \
                 tc.tile_pool(name="rec_psum", bufs=2, space="PSUM") as rpp, \
                 tc.tile_pool(name="rec_psumg", bufs=1, space="PSUM") as rppg, \
                 tc.tile_pool(name="fc_psum", bufs=2, space="PSUM") as fpp:
                whh = {d: wp.tile([P, 4 * 16 * P], F8, name=f"whh{d}", tag=f"whh{d}") for d in dirs}
                wih = {d: wp.tile([P, 2 * 16 * P], F8, name=f"wih{d}", tag=f"wih{d}") for d in dirs}
                brow = {d: wp.tile([1, G], BF16, name=f"brow{d}", tag=f"brow{d}") for d in dirs}
                xemb = wp.tile([P, 2 * T * B], F8, tag="xemb")
                for d in dirs:
                    for k in range(4):
                        dma(whh[d][:, k * 16 * P:(k + 1) * 16 * P],
                            dt[f"whhT_{d}"].ap()[k * P:(k + 1) * P, :])
                        dma(fcw[d][:, k * K:(k + 1) * K],
                            dt[f"fcWT_{d}"].ap()[k * P:(k + 1) * P, :])
                    for k in range(2):
                        dma(wih[d][:, k * 16 * P:(k + 1) * 16 * P],
                            dt[f"wihT_{d}"].ap()[k * P:(k + 1) * P, :])
                    dma(brow[d][:], AP(dt[f"b16_{d}"], 0, [[1, 1], [1, G]]))
                NXQ = 4  # xemb in quarters
                for q in range(NXQ):
                    w = 2 * T * B // NXQ
                    dma(xemb[:, q * w:(q + 1) * w],
                        dt["xembT"].ap()[:, q * w:(q + 1) * w])
                dma(hsf[:, 0:64], dt["h0T"].ap()[:, 0:64])
                dma(hsb[:, 0:64], dt["h0T"].ap()[:, 64:128])
                dma(cgf[:, 0:64], dt["c0T"].ap()[:, 0:64])
                dma(cgb[:, 0:64], dt["c0T"].ap()[:, 64:128])
                dma(fcbs_p[:], AP(dt["fcbs"], 0, [[1, K], [1, 1]]))
                ps_tiles = {}

                def _in_mms(ps, d, blk, m, first):
                    # bias + input-projection matmuls for one m-tile; exactly
                    # one start=True per PSUM bank per step (start zeroes the
                    # whole 2KB zero region)
                    o = (m % 12) * 16 if m < 12 else (m - 12) * 16
                    nc.tensor.matmul(
                        ps[:, o:o + 16],
                        brow[d][0:1, m * P:(m + 1) * P], ones16[0:1, :],
                        start=first, stop=False)
                    nc.tensor.matmul(
                        ps[:, o:o + 16],
                        fv(wih[d], m * P, [[16 * P, 2], [1, P]]),
                        fv(xemb, blk * B, [[TB, 2], [1, B]]),
                        start=False, stop=False, perf_mode=DR)

                def emit_in(t):
                    # per dir: ifo gates and g gates in SEPARATE full-bank
                    # PSUM tiles — PSUM deps are bank-granular, so this lets
                    # tanh(g) start before the ifo matmuls finish
                    pst = {}
                    for d in dirs:
                        psI = rpp.tile([P, 512], F32, name=f"psI{d}", tag=f"psI{d}")
                        psG = rppg.tile([P, 512], F32, name=f"psG{d}", tag=f"psG{d}")
                        pst[d] = (psI, psG)
                        blk = t if d == "f" else T - 1 - t
                        for m in range(12, 16):
                            _in_mms(psG, d, blk, m, m == 12)
                        for m in range(12):
                            _in_mms(psI, d, blk, m, m == 0)
                    ps_tiles[t] = pst

                def whh_mms(pst, d, t):
                    psI, psG = pst[d]
                    rhst = hsf if d == "f" else hsb
                    roff = t * 64
                    for m in list(range(12, 16)) + list(range(12)):
                        ps = psG if m >= 12 else psI
                        o = (m - 12) * 16 if m >= 12 else m * 16
                        last = (m == 15) or (m == 11)
                        for kp in range(2):
                            nc.tensor.matmul(
                                ps[:, o:o + 16],
                                fv(whh[d], (2 * kp * 16 + m) * P, [[16 * P, 2], [1, P]]),
                                fv(rhst, roff + kp * 32, [[16, 2], [1, B]]),
                                start=False, stop=(last and kp == 1), perf_mode=DR)

                maskst = {"ch": None}

                bst = {}

                def elem_f1(pst, t):
                    psI, psG = pst["f"]
                    nc.scalar.activation(cgf[:, 64:128], psG[:, 0:64], TANH)
                    sifo = rp.tile([P, 192], BF16, tag="sifof")
                    nc.scalar.activation(sifo[:], psI[:, 0:192], SIG)
                    return (sifo,)

                def elem_f2(sifo, t):
                    # t12 = [sig_f | sig_i] * [c | tanh(g)] in one op
                    t12 = rp.tile([P, 128], BF16, tag="t12f")
                    nc.vector.tensor_tensor(
                        t12[:], fv(sifo, 64, [[-64, 2], [1, 64]]), cgf[:], op=MUL)
                    nc.vector.tensor_tensor(cgf[:, 0:64], t12[:, 0:64],
                                            t12[:, 64:128], op=ADD)
                    tch = rp.tile([P, 64], BF16, tag="tcf")
                    nc.scalar.activation(tch[:], cgf[:, 0:64], TANH)
                    hslot = hsf[:, (t + 1) * 64: (t + 1) * 64 + 64]
                    nc.vector.tensor_tensor(hslot, sifo[:, 128:192], tch[:], op=MUL)

                def load_masks(t0):
                    n = min(MCH, T - t0)
                    mk = mp.tile([P, MCH * 64], U8, name="maskch", tag="maskch")
                    imk = mp.tile([P, MCH * 64], U8, name="imaskch", tag="imaskch")
                    nc.gpsimd.dma_start(
                        mk[:, 0:n * 64], AP(dt["mask_b"], t0 * P * 64,
                                            [[64, P], [P * 64, n], [1, 64]]))
                    nc.gpsimd.dma_start(
                        imk[:, 0:n * 64], AP(dt["imask_b"], t0 * P * 64,
                                             [[64, P], [P * 64, n], [1, 64]]))
                    maskst[t0] = (mk, imk)

                def elem_b1(pst, t):
                    ps = pst["b"]
                    tl = t % MCH
                    if tl == 0:
                        if t + MCH < T:
                            load_masks(t + MCH)
                        maskst["cur"] = maskst.pop(t)
                    psI, psG = ps
                    nc.scalar.activation(cgb[:, 64:128], psG[:, 0:64], TANH)
                    sifo = rp.tile([P, 192], BF16, tag="sifob")
                    nc.scalar.activation(sifo[:], psI[:, 0:192], SIG)
                    t12 = rp.tile([P, 128], BF16, tag="t12b")
                    nc.vector.tensor_tensor(
                        t12[:], fv(sifo, 64, [[-64, 2], [1, 64]]), cgb[:], op=MUL)
                    cn = rp.tile([P, 64], BF16, tag="cn")
                    nc.vector.tensor_tensor(cn[:], t12[:, 0:64], t12[:, 64:128], op=ADD)
                    mk, imk = maskst["cur"]
                    om = rp.tile([P, 64], BF16, tag="om")
                    nc.vector.tensor_tensor(om[:], sifo[:, 128:192],
                                            mk[:, tl * 64:(tl + 1) * 64], op=MUL)
                    u2 = rp.tile([P, 64], BF16, tag="u2")
                    nc.vector.tensor_tensor(u2[:], hsb[:, t * 64:(t + 1) * 64],
                                            imk[:, tl * 64:(tl + 1) * 64], op=MUL)
                    bst["sifo"], bst["cn"], bst["tl"] = sifo, cn, tl
                    bst["om"], bst["u2"] = om, u2

                def elem_b2(t):
                    sifo, cn, tl = bst["sifo"], bst["cn"], bst["tl"]
                    om, u2 = bst["om"], bst["u2"]
                    tch = rp.tile([P, 64], BF16, tag="tcb")
                    nc.scalar.activation(tch[:], cn[:], TANH)
                    # h_next = mask*sig_o*tanh(c_new) + (1-mask)*h_prev,
                    # written straight into the hs slot (whh_b reads it)
                    b1 = rp.tile([P, 64], BF16, tag="b1")
                    nc.vector.tensor_tensor(b1[:], om[:], tch[:], op=MUL)
                    hslot = hsb[:, (t + 1) * 64: (t + 1) * 64 + 64]
                    nc.vector.tensor_tensor(hslot, b1[:], u2[:], op=ADD)
                    mk = maskst["cur"][0][:, tl * 64:(tl + 1) * 64]
                    nc.vector.copy_predicated(cgb[:, 0:64], mk, cn[:])

                # half-step stagger: dir b's elementwise for step t-1 runs
                # between dir f's matmuls and dir f's elementwise for step t;
                # sig_f/gt_f are emitted before tch_b so the Act queue order
                # matches readiness order
                if "rec" not in SKIP:
                    load_masks(0)
                    emit_in(0)
                for t in range(0 if "rec" in SKIP else T):
                    ps = ps_tiles[t]
                    whh_mms(ps, "f", t)
                    if t > 0:
                        elem_b1(ps_tiles.pop(t - 1), t - 1)
                    sg = elem_f1(ps, t)
                    if t > 0:
                        elem_b2(t - 1)
                    if t + 1 < T:
                        emit_in(t + 1)
                    whh_mms(ps, "b", t)
                    elem_f2(*sg, t)
                if "rec" not in SKIP:
                    elem_b1(ps_tiles.pop(T - 1), T - 1)
                    elem_b2(T - 1)
                # zero vector that depends on the recurrence's last hs write:
                # gates the tail's exp ops so the scheduler cannot hoist them
                # into the recurrence (activation-table thrash)
                zgate = pp.tile([K, 1], F32, tag="zgate")
                nc.vector.tensor_scalar(out=zgate[:], in0=hsb[0:K, T * 64:T * 64 + 1],
                                        scalar1=0.0, scalar2=None, op0=MUL)
                nc.vector.tensor_tensor(fcbs_p[:], fcbs_p[:], zgate[:], op=ADD)

                # fc chunks, emitted in readiness order so they overlap the
                # recurrence tail (chunk c ready at step max(f-end, b-end))
                NCH = max(1, T * B // 512)
                SPC = T // NCH
                order = sorted(range(NCH),
                               key=lambda c: max(SPC * c + SPC - 1, T - 1 - SPC * c))
                for c in (order if "fc" not in SKIP else []):
                    psf = fpp.tile([K, 512], F32, name="psf", tag="psf")
                    for d in dirs:
                        for k in range(4):
                            if d == "f":
                                rhs = fv(hsf, (c * SPC + 1) * 64 + k * 16,
                                         [[64, SPC], [1, B]])
                            else:
                                rhs = fv(hsb, (T - c * SPC) * 64 + k * 16,
                                         [[-64, SPC], [1, B]])
                            nc.tensor.matmul(
                                psf[:, 0:SPC * B], fcw[d][:, k * K:(k + 1) * K], rhs,
                                start=(d == "f" and k == 0), stop=(d == "b" and k == 3))
                    sl = slice(c * SPC * B, (c + 1) * SPC * B)
                    nc.vector.tensor_copy(emisT[:, sl], psf[:, 0:SPC * B])

            # ---- CRF chain, gold, capture ----
            with tc.tile_pool(name="tail_sbuf", bufs=2) as cp, \
                 tc.tile_pool(name="tail_persist", bufs=1) as cpr, \
                 tc.tile_pool(name="tail_psum", bufs=2, space="PSUM") as cpp:
                transTs = cpr.tile([K, K], F32, tag="transTs")
                nc.gpsimd.dma_start(transTs[:], dt["transT"].ap()[:])
                ET = cpr.tile([K, K], F32, tag="ET")
                nc.scalar.activation(ET[:], transTs[:], EXP, bias=zgate[:, 0:1])
                Estop = cpr.tile([K, 1], F32, tag="Estop")
                nc.scalar.activation(Estop[:], transTs[:, STOP:STOP + 1], EXP,
                                     bias=zgate[:, 0:1])
                ones12 = cpr.tile([K, K], F32, tag="ones12")
                nc.gpsimd.memset(ones12[:], 1.0)
                NCH = max(1, T * B // 512)
                SPC = T // NCH
                for c in range(NCH):
                    sl = slice(c * SPC * B, (c + 1) * SPC * B)
                    nc.scalar.activation(expem[:, sl], emisT[:, sl], EXP,
                                         bias=fcbs_p[:, 0:1])
                a0 = cpr.tile([K, B], F32, tag="a0")
                nc.gpsimd.dma_start(a0[:], dt["a0"].ap()[:])
                Lh = cpr.tile([1, NE * B], F32, tag="Lh")
                nc.gpsimd.memset(Lh[:], 0.0)
                selb = cpr.tile([K, T * B], F32, tag="selb")
                nc.gpsimd.dma_start(selb[:], dt["sel"].ap()[:])
                mselb = cpr.tile([K, T * B], F32, tag="mselb")
                nc.gpsimd.dma_start(mselb[:], dt["msel"].ap()[:])

                if DEBUG:
                    nc.gpsimd.dma_start(demis_o.ap()[:], emisT[:])
                    nc.gpsimd.dma_start(
                        AP(dhs_o, 0, [[(T + 1) * 128, P], [1, (T + 1) * 64]]), hsf[:])
                    nc.gpsimd.dma_start(
                        AP(dhs_o, (T + 1) * 64, [[(T + 1) * 128, P], [1, (T + 1) * 64]]), hsb[:])

                # CRF forward chain; renorm factors are computed off-chain
                # and applied one step late so the chain never stalls
                rhs = a0[:, 0:B]
                rS_pend = None
                for t in range(0 if "crf" in SKIP else T):
                    blk = expem[:, t * B:(t + 1) * B]
                    psc = cpp.tile([K, 512], F32, name="psc", tag="psc")
                    nc.tensor.matmul(psc[:, 0:B], ET[:], rhs, start=True, stop=True)
                    nc.vector.tensor_tensor(blk, psc[:, 0:B], blk, op=MUL)
                    if t % R == 0 and t > 0:
                        nc.gpsimd.tensor_tensor(blk, blk, rS_pend[:], op=MUL)
                    rhs = blk
                    if t % R == R - 1 and t < T - 1:
                        j = (t + 1) // R
                        pss = cpp.tile([K, 512], F32, name="pss", tag="pss", bufs=2)
                        nc.tensor.matmul(pss[:, 0:B], ones12[:], blk,
                                         start=True, stop=True)
                        rS_pend = cp.tile([K, B], F32, tag="rS")
                        nc.vector.reciprocal(rS_pend[:], pss[:, 0:B])
                        lnS = cp.tile([1, B], F32, tag="lnS")
                        nc.scalar.activation(lnS[:], pss[0:1, 0:B], LN)
                        nc.vector.tensor_tensor(Lh[:, j * B:(j + 1) * B],
                                                Lh[:, (j - 1) * B:j * B],
                                                lnS[:], op=ADD)

                # gold score (independent of the chain; fills engine gaps)
                tfl = cp.tile([1, 144], F32, tag="tfl")
                nc.gpsimd.dma_start(tfl[:], AP(dt["trans"], 0, [[1, 1], [1, 144]]))
                tfb = cp.tile([B, 144], F32, tag="tfb")
                nc.gpsimd.partition_broadcast(tfb[:], tfl[:])
                cnts = cp.tile([B, 144], F32, tag="cnts")
                nc.gpsimd.dma_start(cnts[:], dt["counts"].ap()[:])
                pr1 = cp.tile([B, 144], F32, tag="pr1")
                nc.vector.tensor_tensor(pr1[:], cnts[:], tfb[:], op=MUL)
                g1 = cp.tile([B, 1], F32, tag="g1")
                nc.vector.tensor_reduce(g1[:], pr1[:], axis=X, op=ADD)
                fcbr = cp.tile([1, K], F32, tag="fcbr")
                nc.gpsimd.dma_start(fcbr[:], AP(dt["fcb"], 0, [[1, 1], [1, K]]))
                fcbb = cp.tile([B, K], F32, tag="fcbb")
                nc.gpsimd.partition_broadcast(fcbb[:], fcbr[:])
                cntbs = cp.tile([B, K], F32, tag="cntbs")
                nc.gpsimd.dma_start(cntbs[:], dt["cntb"].ap()[:])
                pr2 = cp.tile([B, K], F32, tag="pr2")
                nc.vector.tensor_tensor(pr2[:], cntbs[:], fcbb[:], op=MUL)
                g2 = cp.tile([B, 1], F32, tag="g2")
                nc.vector.tensor_reduce(g2[:], pr2[:], axis=X, op=ADD)
                g12 = cp.tile([B, 1], F32, tag="g12")
                nc.vector.tensor_tensor(g12[:], g1[:], g2[:], op=ADD)
                nc.gpsimd.dma_start(AP(scr16, 0, [[1, B], [1, 1]]), g12[:])
                g12r = cp.tile([1, B], F32, tag="g12r")
                nc.gpsimd.dma_start(g12r[:], AP(scr16, 0, [[1, 1], [1, B]]))
                selp = cpr.tile([K, NCH * B], F32, tag="selp")
                for c in range(NCH):
                    sl = slice(c * SPC * B, (c + 1) * SPC * B)
                    nc.vector.tensor_tensor(selb[:, sl], emisT[:, sl],
                                            selb[:, sl], op=MUL)
                    nc.vector.tensor_reduce(
                        selp[:, c * B:(c + 1) * B],
                        fv(selb, c * SPC * B, [[1, B], [B, SPC]]), axis=X, op=ADD)
                g3 = cp.tile([K, B], F32, tag="g3")
                nc.vector.tensor_reduce(g3[:], fv(selp, 0, [[1, B], [B, NCH]]),
                                        axis=X, op=ADD)
                ps3 = cpp.tile([K, 512], F32, name="ps3", tag="pss", bufs=2)
                nc.tensor.matmul(ps3[:, 0:B], ones12[:], g3[:], start=True, stop=True)
                goldT = cp.tile([1, B], F32, tag="goldT")
                nc.vector.tensor_tensor(goldT[:], g12r[:], ps3[0:1, 0:B], op=ADD)
                nc.gpsimd.dma_start(AP(dgold_o, 0, [[1, 1], [1, B]]), goldT[:])

                # capture at t = len-1 (expem holds the alpha history);
                # chunked so it runs as the chain passes each block
                aendp = cpr.tile([K, NCH * B], F32, tag="aendp")
                for c in range(NCH):
                    sl = slice(c * SPC * B, (c + 1) * SPC * B)
                    nc.vector.tensor_tensor(expem[:, sl], expem[:, sl],
                                            mselb[:, sl], op=MUL)
                    nc.vector.tensor_reduce(
                        aendp[:, c * B:(c + 1) * B],
                        fv(expem, c * SPC * B, [[1, B], [B, SPC]]), axis=X, op=ADD)
                aend = cp.tile([K, B], F32, tag="aend")
                nc.vector.tensor_reduce(aend[:], fv(aendp, 0, [[1, B], [B, NCH]]),
                                        axis=X, op=ADD)
                mep = cp.tile([1, NE * B], F32, tag="mep")
                nc.gpsimd.dma_start(mep[:], AP(dt["maskep"], 0, [[1, 1], [1, NE * B]]))
                prod5 = cp.tile([1, NE * B], F32, tag="prod5")
                nc.vector.tensor_tensor(prod5[:], Lh[:], mep[:], op=MUL)
                Lend = cp.tile([1, B], F32, tag="Lend")
                nc.vector.tensor_reduce(Lend[:], fv(prod5, 0, [[1, B], [B, NE]]),
                                        axis=X, op=ADD)
                azs = cp.tile([K, B], F32, tag="azs")
                nc.vector.tensor_scalar(out=azs[:], in0=aend[:], scalar1=Estop[:, 0:1],
                                        scalar2=None, op0=MUL)
                ps2 = cpp.tile([K, 512], F32, name="ps2", tag="pss", bufs=2)
                nc.tensor.matmul(ps2[:, 0:B], ones12[:], azs[:], start=True, stop=True)
                logz0 = cp.tile([1, B], F32, tag="logz0")
                nc.scalar.activation(logz0[:], ps2[0:1, 0:B], LN)
                corr = cp.tile([1, B], F32, tag="corr")
                nc.gpsimd.dma_start(corr[:], AP(dt["corrL"], 0, [[1, 1], [1, B]]))
                logzp = cp.tile([1, B], F32, tag="logzp")
                nc.vector.tensor_tensor(logzp[:], logz0[:], Lend[:], op=ADD)
                logzf = cp.tile([1, B], F32, tag="logzf")
                nc.vector.tensor_tensor(logzf[:], logzp[:], corr[:], op=ADD)
                nc.gpsimd.dma_start(AP(dlogz_o, 0, [[1, 1], [1, B]]), logzf[:])
                nllT = cp.tile([1, B], F32, tag="nllT")
                nc.vector.tensor_tensor(nllT[:], logzf[:], goldT[:], op=SUB)
                nc.gpsimd.dma_start(AP(nll_o, 0, [[1, 1], [1, B]]), nllT[:])
    return nc


_CACHE = {}


def get_program():
    if "nc" not in _CACHE:
        nc = bacc.Bacc("TRN2", target_bir_lowering=False, debug=False,
                       num_devices=NCORES)
        build(nc)
        nc.compile()
        _CACHE["nc"] = nc
    return _CACHE["nc"]


def perm_ifog(w):
    # [4H, ...] rows i,f,g,o -> i,f,o,g
    return np.concatenate([w[0:512], w[512:1024], w[1536:2048], w[1024:1536]], 0)


def host_prep(inputs):
    f32 = np.float32
    bf = ml_dtypes.bfloat16
    f8 = ml_dtypes.float8_e4m3fn
    x = np.asarray(inputs["x"]).astype(np.int64)
    lengths = np.asarray(inputs["lengths"]).astype(np.int64)
    tags = np.asarray(inputs["tags"]).astype(np.int64)
    emb = np.asarray(inputs["embedding"], f32)
    trans = np.asarray(inputs["trans"], f32)
    fcW = np.asarray(inputs["fc_W"], f32)
    fcb = np.asarray(inputs["fc_b"], f32)
    h0 = np.asarray(inputs["h0"], f32)
    c0 = np.asarray(inputs["c0"], f32)

    Wd, Brow = {}, {}
    for d in ("f", "b"):
        wih = perm_ifog(np.asarray(inputs[f"W_ih_{d}"], f32))
        whh = perm_ifog(np.asarray(inputs[f"W_hh_{d}"], f32))
        bi = perm_ifog(np.asarray(inputs[f"b_ih_{d}"], f32)[:, None])[:, 0]
        bh = perm_ifog(np.asarray(inputs[f"b_hh_{d}"], f32)[:, None])[:, 0]
        Wd[d] = (wih.T.astype(f8).copy(), whh.T.astype(f8).copy())
        Brow[d] = (bi + bh).reshape(16, P).astype(bf).copy()
    onehot = (np.arange(256)[None, :] // 16 == np.arange(16)[:, None]).astype(bf)

    fcWT = {"f": fcW[:, :H].T.astype(f8).copy(), "b": fcW[:, H:].T.astype(f8).copy()}
    emb_bf = emb.astype(f8)

    maps = []
    for c in range(NCORES):
        bs = slice(c * B, (c + 1) * B)
        xs = x[bs]            # [16, T]
        ln = lengths[bs]      # [16]
        tg = tags[bs]         # [16, T]
        m = {"trans": trans, "transT": trans.T.astype(f32).copy(),
             "fcb": fcb, "fcbs": (fcb - SHIFT).astype(f32),
             "corrL": (SHIFT * ln).astype(f32)}
        # xembT: [256, T, 16] -> [2, 128, T*16] -> [128, 2*T*16]
        e = emb_bf[xs]                        # [16, T, 256]
        eT = e.transpose(2, 1, 0)             # [256, T, 16]
        arr = eT.reshape(2, P, T * B)
        m["xembT"] = np.ascontiguousarray(
            np.concatenate([arr[0], arr[1]], axis=1))
        m["onehot"] = onehot
        for d in ("f", "b"):
            m[f"wihT_{d}"], m[f"whhT_{d}"] = Wd[d]
            m[f"b16_{d}"] = Brow[d]
            m[f"fcWT_{d}"] = fcWT[d]
        h0T = np.concatenate(
            [h0[di, bs].T.reshape(4, P, B).transpose(1, 0, 2).reshape(P, 64)
             for di in (0, 1)], axis=1)
        c0T = np.concatenate(
            [c0[di, bs].T.reshape(4, P, B).transpose(1, 0, 2).reshape(P, 64)
             for di in (0, 1)], axis=1)
        m["h0T"] = h0T.astype(f8).copy()
        m["c0T"] = c0T.astype(bf).copy()
        # bwd mask: step s processes tau = T-1-s; valid iff tau < len
        tau = (T - 1 - np.arange(T))[:, None]          # [T, 1]
        mk = (tau < ln[None, :]).astype(f32)           # [T, 16]
        mku8 = np.broadcast_to(
            mk[:, None, None, :], (T, P, 4, B)).reshape(T, P, 64).astype(np.uint8)
        m["mask_b"] = mku8.copy()
        m["imask_b"] = (1 - mku8).copy()
        a0 = np.zeros((K, B), f32); a0[START, :] = 1.0
        m["a0"] = a0
        msel = np.zeros((K, T, B), f32)
        msel[:, ln - 1, np.arange(B)] = 1.0
        m["msel"] = msel.reshape(K, T * B)
        mep = np.zeros((NE, B), f32)
        mep[(ln - 1) // R, np.arange(B)] = 1.0
        m["maskep"] = mep.reshape(-1)
        tarange = np.arange(T)[None, :]
        valid = tarange < ln[:, None]                  # [16, T]
        selm = np.zeros((K, T, B), f32)
        jj = np.arange(K)[:, None, None]
        selm[:] = (tg.T[None] == jj) & valid.T[None]
        m["sel"] = np.ascontiguousarray(selm.reshape(K, T * B))
        counts = np.zeros((B, 144), f32)
        cntb = np.zeros((B, K), f32)
        for b in range(B):
            L = int(ln[b])
            prev = START
            for t in range(L):
                nx = int(tg[b, t])
                counts[b, nx * K + prev] += 1
                cntb[b, nx] += 1
                prev = nx
            counts[b, STOP * K + prev] += 1
        m["counts"] = counts
        m["cntb"] = cntb
        maps.append(m)
    return maps


def kernel(**inputs):
    from concourse.bass_utils import run_bass_kernel_spmd
    nc = get_program()
    maps = host_prep(inputs)
    res = run_bass_kernel_spmd(nc, maps, core_ids=list(range(NCORES)))
    out = np.concatenate([r["nll"] for r in res.results]).astype(np.float32)
    kernel.last_results = res
    return out
